# revision 28
# baseline (speedup 1.0000x reference)
"""MinLSTM layer on 8 Trainium2 NeuronCores.

Math (equivalent to the log-space reference, done in linear space):
    f_pre = x @ W_f.T + b_f ; i_pre = x @ W_i.T + b_i ; h_pre = x @ W_h.T + b_h
    sf = sigmoid(f_pre) ; si = sigmoid(i_pre)
    f = sf / (sf + si)                       # normalized forget gate
    i = 1 - f                                # = si / (sf + si)
    g = max(sigmoid(h_pre), h_pre + 0.5)     # == exp(log_g), exactly
    h_t = f_t * h_{t-1} + i_t * g_t,  h_0 = 1
The gates satisfy f in (0,1), g > 0, so h stays in a tame range and the
recurrence is numerically stable in fp32.

Sharding: 8 cores = batch(4) x hidden-halves(2). Core c handles batch b=c//2,
hidden slice [(c%2)*512, (c%2+1)*512). No cross-core communication; the scan
runs along T inside each core via the DVE TensorTensorScan instruction
(state = f*state - mv per step, mv = (f-1)*g = -i*g).

Device layout: gates computed as [h_part, t_free] via out = W_sliceT.T @ xT;
host pre-transposes x and W (numpy, bf16) and re-transposes the [512, 4096]
per-core output back to [T, Dh]. Matmuls run in 512-wide t-chunks (one PSUM
bank); elementwise+scan run in up-to-1024-wide super-chunks.

Matmul operands are bf16 (max rel err vs the fp32 reference ~7e-3, within
the 2e-2 budget; fp32r would be ~6e-4 but streams 13ns/matmul slower and
doubles DMA traffic).

Engine use: PE matmuls at ~216ns/512-col group; ACT does the sigmoids off
PSUM; DVE does g-max, normalization, mv, and the scan; GPSIMD (software
ALU, ~3us/1024-elem op, no PSUM access, TensorTensor only) takes just the
dependency-head add s=sf+si on 1024-chunks -- anything GPSIMD mid-chain
backpressures the tile pools and stalls the PE.

Startup: DMA descriptor issue costs ~605ns each on an engine queue, so the
early supply is ISSUE-bound, not bandwidth-bound. x tiles go one descriptor
per 512-col group ([128, 8*512] batched k-slabs) on the Sync queue, except
the first group which stays per-k so the first matmul depends on only
128KB. W_f issues per-k on the Scalar queue in parallel; W_i/W_h are one
descriptor each on the GPSIMD queue. Warmup matmuls on a zeroed scratch
tile cover the preamble->first-data window at full array duty, keeping the
PE's HAM activity monitor ramping (N=128-wide warmups never trip it; an
idle gap resets it and delays the 2.4GHz grant by several us).

Tail: the last two 512-chunks are paired per h-tile (h-tile i finishes its
whole T range before h-tile i+1's final chunks), f/i gates and the
normalization run before the h-gate matmuls, and the very last unit's
f-phase is hoisted before its sibling's h-phase: only sigmoid -> g-max ->
mv -> scan -> store trails the final matmul.
"""

import sys

for _p in ("/opt/trn_rl_repo",):
    if _p not in sys.path:
        sys.path.append(_p)

import numpy as np
import ml_dtypes

import concourse.bass as bass
import concourse.tile as tile
from concourse import bacc, mybir
from concourse.bass_utils import run_bass_kernel_spmd

B, T, DIN, DH = 4, 4096, 1024, 1024
N_CORES = 8
HSH = DH // 2          # 512 hidden channels per core
P = 128                # partitions
KT = DIN // P          # 8 contraction tiles
NT = 512               # matmul t-chunk (free dim, one PSUM bank)
IT = HSH // P          # 4 h-tiles per core
# elementwise/scan super-chunks (start, length)
CHUNKS = [(0, 1024), (1024, 1024), (2048, 1024), (3072, 512), (3584, 512)]
N_WARM = 9             # warmup matmuls: bridge preamble end (~7.4us) to
                       # first-data (~11.6us) with no idle gap (a gap
                       # resets the HAM clock ramp)

MM_DT = mybir.dt.bfloat16
NP_MM_DT = ml_dtypes.bfloat16

_COMPILED = None


def _build():
    AF = mybir.ActivationFunctionType
    OP = mybir.AluOpType
    f32 = mybir.dt.float32

    nc = bacc.Bacc("TRN2", target_bir_lowering=False, debug=False)

    xT = nc.dram_tensor("xT", [DIN, T], MM_DT, kind="ExternalInput").ap()
    wd = {g: nc.dram_tensor(f"w{g}", [DIN, HSH], MM_DT, kind="ExternalInput").ap()
          for g in ("f", "i", "h")}
    # packed per-partition scalars: [b_f | b_i | b_h | b_h+0.5], each (128, IT)
    biases = nc.dram_tensor("biases", [P, 4 * IT], f32, kind="ExternalInput").ap()
    out = nc.dram_tensor("out", [HSH, T], f32, kind="ExternalOutput").ap()

    # DRAM views: (KT*P, n) -> [p, k, n]
    xT_v = xT.rearrange("(k p) t -> p k t", p=P)
    w_v = {g: w.rearrange("(k p) h -> p k h", p=P) for g, w in wd.items()}

    with tile.TileContext(nc) as tc:
        with (
            tc.tile_pool(name="wpool", bufs=1) as wpool,
            tc.tile_pool(name="bpool", bufs=1) as bpool,
            tc.tile_pool(name="xpool", bufs=8) as xpool,
            tc.tile_pool(name="psum", bufs=8, space="PSUM") as pspool,
            tc.tile_pool(name="work", bufs=4) as work,
            tc.tile_pool(name="hpool", bufs=6) as hpool,
        ):
            bias_t = bpool.tile([P, 4 * IT], f32, tag="bias")

            # all W per-k: consumption is k-ordered, so per-k 128KB tiles
            # keep the DMA->matmul dependency just-in-time (a batched 1MB
            # descriptor completes after its first consumer needs it)
            wt = {g: [wpool.tile([P, HSH], MM_DT, tag=f"w{g}{k}", name=f"w{g}{k}_t")
                      for k in range(KT)] for g in ("f", "i", "h")}

            def w_ap(g, k, hsl):
                return wt[g][k][:, hsl]

            def bias_ap(kind, i):
                return bias_t[:, kind * IT + i:kind * IT + i + 1]

            def fchain(sf, si, ne):
                """Normalize: s=sf+si, r=1/s, f=sf*r. In-place: sf becomes f.

                The add goes to GPSIMD only on big chunks: it is the
                dependency head (fed straight by the ACT sigmoids), so its
                software latency hides; everything downstream of the
                reciprocal stays on the DVE.
                """
                s = work.tile([P, ne], f32, tag="s", name="s_t")
                eng = nc.gpsimd if ne >= 1024 else nc.vector
                eng.tensor_add(s[:], sf[:, :ne], si[:, :ne])
                r = work.tile([P, ne], f32, tag="r", name="r_t")
                nc.vector.reciprocal_approx_fast(out=r[:], in_=s[:])
                nc.vector.tensor_mul(sf[:, :ne], sf[:, :ne], r[:])

            def hchain(i, f, gt, J, t0, ne):
                """mv=(f-1)*g, scan state=f*state-mv, store chunk."""
                nc.vector.scalar_tensor_tensor(
                    gt[:, :ne], f[:, :ne], 1.0, gt[:, :ne],
                    op0=OP.subtract, op1=OP.mult)
                hc = hpool.tile([P, ne], f32, tag="h", name=f"h{i}_t")
                init = 1.0 if J == 0 else hprev[i][:, -1:]
                nc.vector.tensor_tensor_scan(
                    hc[:], f[:, :ne], gt[:, :ne], init,
                    op0=OP.mult, op1=OP.subtract)
                hprev[i] = hc
                nc.sync.dma_start(
                    out=out[i * P:(i + 1) * P, t0:t0 + ne], in_=hc[:])

            hprev = [None] * IT
            hsls = [slice(i * P, (i + 1) * P) for i in range(IT)]

            # Warmups on a zeroed scratch tile keep PE duty high from
            # preamble end until the first real data lands.
            scratch = bpool.tile([P, NT], MM_DT, tag="scratch")
            nc.vector.memset(scratch[:].bitcast(mybir.dt.uint16), 0)
            pswarm = pspool.tile([P, NT], f32, tag="ps", name="pswarm_t")
            for _ in range(N_WARM):
                nc.tensor.matmul(pswarm[:], lhsT=scratch[:, :P], rhs=scratch[:],
                                 start=True, stop=True)

            # ---- DMA issue: ALL early DMAs on the sync queue in strict
            # priority order. Issue costs ~605ns/descriptor and the early
            # hardware DMA bandwidth is scarce: anything issued from another
            # engine queue (or out of order) steals bandwidth from the
            # (x, W_f) stream the first matmuls wait on and delays the HAM
            # clock ramp by several us.
            xk0 = [xpool.tile([P, NT], MM_DT, tag="xk", name="xk_t", bufs=16)
                   for _ in range(KT)]
            for k in range(KT):
                nc.sync.dma_start(out=xk0[k][:], in_=xT_v[:, k, 0:NT])
                nc.sync.dma_start(out=wt["f"][k][:], in_=w_v["f"][:, k, :])
                if k == 0:
                    nc.sync.dma_start(out=bias_t[:], in_=biases[:])
            xk1 = [xpool.tile([P, NT], MM_DT, tag="xk", name="xk_t", bufs=16)
                   for _ in range(KT)]
            for k in range(KT):
                nc.sync.dma_start(out=xk1[k][:], in_=xT_v[:, k, NT:2 * NT])
            for g in ("i", "h"):
                for k in range(KT):
                    nc.sync.dma_start(out=wt[g][k][:], in_=w_v[g][:, k, :])

            def rhs0(k):
                return xk0[k][:]

            def rhs1(k):
                return xk1[k][:]

            # ---- J0: gate-major, k-outer; PE chases the DMA stream ----
            t0, ne = CHUNKS[0]
            sf = [work.tile([P, ne], f32, tag="sf", name="sf_t") for _ in range(IT)]
            si = [work.tile([P, ne], f32, tag="si", name="si_t") for _ in range(IT)]
            sg = [work.tile([P, ne], f32, tag="sg", name="sg_t") for _ in range(IT)]
            gt = [work.tile([P, ne], f32, tag="gt", name="gt_t") for _ in range(IT)]
            for gate, dsts, bk in (("f", sf, 0), ("i", si, 1), ("h", sg, 2)):
                for half, rhsf in enumerate((rhs0, rhs1)):
                    esl = slice(half * NT, (half + 1) * NT)
                    psts = [pspool.tile([P, NT], f32, tag="ps", name="ps_t")
                            for _ in range(IT)]
                    for k in range(KT):
                        for pst, hsl in zip(psts, hsls):
                            nc.tensor.matmul(
                                pst[:], lhsT=w_ap(gate, k, hsl), rhs=rhsf(k),
                                start=(k == 0), stop=(k == KT - 1))
                    for i in range(IT):
                        nc.scalar.activation(dsts[i][:, esl], psts[i][:], AF.Sigmoid,
                                             bias=bias_ap(bk, i), scale=1.0)
                        if gate == "h":
                            # DVE: GPSIMD cannot read PSUM
                            nc.vector.scalar_tensor_tensor(
                                gt[i][:, esl], psts[i][:], bias_ap(3, i),
                                sg[i][:, esl], op0=OP.add, op1=OP.max)
            for i in range(IT):
                fchain(sf[i], si[i], ne)
                hchain(i, sf[i], gt[i], 0, t0, ne)

            # ---- J1+: h-tile-major units ----
            def load_xcs(t0, ne):
                """One batched [P, KT*w] tile (single descriptor) per
                column group."""
                nfull, rem = divmod(ne, NT)
                xcs = []
                toff = t0
                for w_ in [NT] * nfull + ([rem] if rem else []):
                    xt = xpool.tile([P, KT * w_], MM_DT, tag="x8", name="x8_t",
                                    bufs=4)
                    nc.sync.dma_start(out=xt[:], in_=xT_v[:, :, toff:toff + w_])
                    xcs.append((xt, toff - t0, w_))
                    toff += w_
                return xcs

            def mm_group(gate, hsl, xt, w_):
                """K-accumulated matmul group -> PSUM tile (full bank: PSUM
                zeroing on start=True is 2KB-bank granular)."""
                pst = pspool.tile([P, NT], f32, tag="ps", name="ps_t")
                for k in range(KT):
                    nc.tensor.matmul(
                        pst[:, :w_], lhsT=w_ap(gate, k, hsl),
                        rhs=xt[:, k * w_:(k + 1) * w_],
                        start=(k == 0), stop=(k == KT - 1))
                return pst

            def unit_f(i, ne, xcs):
                """f/i gates + normalization for one (h-tile, chunk)."""
                sf = work.tile([P, ne], f32, tag="sf", name="sf_t")
                si = work.tile([P, ne], f32, tag="si", name="si_t")
                for xt, eoff, w_ in xcs:
                    esl = slice(eoff, eoff + w_)
                    for gate, dst, bk in (("f", sf, 0), ("i", si, 1)):
                        pst = mm_group(gate, hsls[i], xt, w_)
                        nc.scalar.activation(
                            dst[:, esl], pst[:, :w_], AF.Sigmoid,
                            bias=bias_ap(bk, i), scale=1.0)
                fchain(sf, si, ne)
                return sf

            def unit_h(i, J, t0, ne, xcs, sf):
                """h gate, g-max, scan, store for one (h-tile, chunk)."""
                sg = work.tile([P, ne], f32, tag="sg", name="sg_t")
                gt = work.tile([P, ne], f32, tag="gt", name="gt_t")
                for xt, eoff, w_ in xcs:
                    esl = slice(eoff, eoff + w_)
                    pst = mm_group("h", hsls[i], xt, w_)
                    nc.scalar.activation(
                        sg[:, esl], pst[:, :w_], AF.Sigmoid,
                        bias=bias_ap(2, i), scale=1.0)
                    # DVE: GPSIMD cannot read PSUM
                    nc.vector.scalar_tensor_tensor(
                        gt[:, esl], pst[:, :w_], bias_ap(3, i),
                        sg[:, esl], op0=OP.add, op1=OP.max)
                hchain(i, sf, gt, J, t0, ne)

            def unit(i, J, t0, ne, xcs):
                unit_h(i, J, t0, ne, xcs, unit_f(i, ne, xcs))

            # middle 1024-chunks: h-tile-major
            for J, (t0, ne) in enumerate(CHUNKS[1:-2], start=1):
                xcs = load_xcs(t0, ne)
                for i in range(IT):
                    unit(i, J, t0, ne, xcs)

            # final two 512-chunks, paired per h-tile so the end-of-kernel
            # drain is a single unit's h-chain; the very last unit's f-phase
            # is hoisted ahead of its sibling's h-phase
            JA, JB = len(CHUNKS) - 2, len(CHUNKS) - 1
            (tA, neA), (tB, neB) = CHUNKS[JA], CHUNKS[JB]
            xcsA = load_xcs(tA, neA)
            xcsB = load_xcs(tB, neB)
            for i in range(IT - 1):
                unit(i, JA, tA, neA, xcsA)
                unit(i, JB, tB, neB, xcsB)
            iL = IT - 1
            sfA = unit_f(iL, neA, xcsA)
            sfB = unit_f(iL, neB, xcsB)
            unit_h(iL, JA, tA, neA, xcsA, sfA)
            unit_h(iL, JB, tB, neB, xcsB, sfB)

    nc.compile()
    return nc


def _in_maps(x, W_f, b_f, W_i, b_i, W_h, b_h):
    x = np.asarray(x, np.float32)
    wT = {g: np.ascontiguousarray(np.asarray(w, np.float32).T).astype(NP_MM_DT)
          for g, w in (("f", W_f), ("i", W_i), ("h", W_h))}
    bs = {g: np.asarray(b, np.float32) for g, b in (("f", b_f), ("i", b_i), ("h", b_h))}

    maps = []
    for c in range(N_CORES):
        b, hh = divmod(c, 2)
        hsl = slice(hh * HSH, (hh + 1) * HSH)
        bias_pack = np.concatenate([
            bs["f"][hsl].reshape(IT, P).T,
            bs["i"][hsl].reshape(IT, P).T,
            bs["h"][hsl].reshape(IT, P).T,
            (bs["h"][hsl] + 0.5).reshape(IT, P).T,
        ], axis=1)
        maps.append({
            "xT": np.ascontiguousarray(x[b].T).astype(NP_MM_DT),
            "wf": np.ascontiguousarray(wT["f"][:, hsl]),
            "wi": np.ascontiguousarray(wT["i"][:, hsl]),
            "wh": np.ascontiguousarray(wT["h"][:, hsl]),
            "biases": np.ascontiguousarray(bias_pack, dtype=np.float32),
        })
    return maps


def kernel(x, W_f, b_f, W_i, b_i, W_h, b_h):
    global _COMPILED
    if _COMPILED is None:
        _COMPILED = _build()
    nc = _COMPILED

    res = run_bass_kernel_spmd(
        nc, _in_maps(x, W_f, b_f, W_i, b_i, W_h, b_h), list(range(N_CORES)))

    full = np.empty((B, T, DH), np.float32)
    for c in range(N_CORES):
        b, hh = divmod(c, 2)
        full[b, :, hh * HSH:(hh + 1) * HSH] = res.results[c]["out"].T
    return full


# revision 29
# speedup vs baseline: 1.0213x; 1.0213x over previous
"""MinLSTM layer on 8 Trainium2 NeuronCores.

Math (equivalent to the log-space reference, done in linear space):
    f_pre = x @ W_f.T + b_f ; i_pre = x @ W_i.T + b_i ; h_pre = x @ W_h.T + b_h
    sf = sigmoid(f_pre) ; si = sigmoid(i_pre)
    f = sf / (sf + si)                       # normalized forget gate
    i = 1 - f                                # = si / (sf + si)
    g = max(sigmoid(h_pre), h_pre + 0.5)     # == exp(log_g), exactly
    h_t = f_t * h_{t-1} + i_t * g_t,  h_0 = 1
The gates satisfy f in (0,1), g > 0, so h stays in a tame range and the
recurrence is numerically stable in fp32.

Sharding: 8 cores = batch(4) x hidden-halves(2). Core c handles batch b=c//2,
hidden slice [(c%2)*512, (c%2+1)*512). No cross-core communication; the scan
runs along T inside each core via the DVE TensorTensorScan instruction
(state = f*state - mv per step, mv = (f-1)*g = -i*g).

Device layout: gates computed as [h_part, t_free] via out = W_sliceT.T @ xT;
host pre-transposes x and W (numpy, bf16) and re-transposes the [512, 4096]
per-core output back to [T, Dh]. Matmuls run in 512-wide t-chunks (one PSUM
bank); elementwise+scan run in up-to-1024-wide super-chunks.

Matmul operands are bf16 (max rel err vs the fp32 reference ~7e-3, within
the 2e-2 budget; fp32r would be ~6e-4 but streams 13ns/matmul slower and
doubles DMA traffic).

Engine use: PE matmuls at ~216ns/512-col group; ACT does the sigmoids off
PSUM; DVE does g-max, normalization, mv, and the scan; GPSIMD (software
ALU, ~3us/1024-elem op, no PSUM access, TensorTensor only) takes just the
dependency-head add s=sf+si on 1024-chunks -- anything GPSIMD mid-chain
backpressures the tile pools and stalls the PE.

Startup: DMA descriptor issue costs ~605ns each, so the early supply is
ISSUE-bound, not bandwidth-bound. ALL early DMAs go on the single Sync
queue in strict priority order -- (x0[k], W_f[k]) pairs, bias, x1[k], then
W_i/W_h per-k. Per-k 128KB tiles keep the DMA->matmul dependency
just-in-time (consumption is k-ordered; a batched 1MB descriptor finishes
after its first consumer needs it). Issuing W from another engine queue
steals early DMA bandwidth from the critical pair stream and stalls the
first real matmuls by ~6us (measured), which also drops the HAM clock.
J1+ x tiles are one descriptor per 512-col group ([128, 8*512] batched
k-slabs) -- by then DMA is far ahead. Warmup matmuls on a zeroed scratch
tile cover the preamble->first-data window at full array duty, keeping the
PE's HAM activity monitor ramping (N=128-wide warmups never trip it; an
idle gap resets it and delays the 2.4GHz grant by several us).

Tail: the last two 512-chunks are paired per h-tile (h-tile i finishes its
whole T range before h-tile i+1's final chunks), f/i gates and the
normalization run before the h-gate matmuls, and the very last unit's
f-phase is hoisted before its sibling's h-phase: only sigmoid -> g-max ->
mv -> scan -> store trails the final matmul.
"""

import sys

for _p in ("/opt/trn_rl_repo",):
    if _p not in sys.path:
        sys.path.append(_p)

import numpy as np
import ml_dtypes

import concourse.bass as bass
import concourse.tile as tile
from concourse import bacc, mybir
from concourse.bass_utils import run_bass_kernel_spmd

B, T, DIN, DH = 4, 4096, 1024, 1024
N_CORES = 8
HSH = DH // 2          # 512 hidden channels per core
P = 128                # partitions
KT = DIN // P          # 8 contraction tiles
NT = 512               # matmul t-chunk (free dim, one PSUM bank)
IT = HSH // P          # 4 h-tiles per core
# elementwise/scan super-chunks (start, length)
CHUNKS = [(0, 1024), (1024, 1024), (2048, 1024), (3072, 512), (3584, 512)]
N_WARM = 9             # warmup matmuls: bridge preamble end (~7.4us) to
                       # first-data (~11.6us) with no idle gap (a gap
                       # resets the HAM clock ramp)

MM_DT = mybir.dt.bfloat16
NP_MM_DT = ml_dtypes.bfloat16

_COMPILED = None


def _build():
    AF = mybir.ActivationFunctionType
    OP = mybir.AluOpType
    f32 = mybir.dt.float32

    nc = bacc.Bacc("TRN2", target_bir_lowering=False, debug=False)

    xT = nc.dram_tensor("xT", [DIN, T], MM_DT, kind="ExternalInput").ap()
    wd = {g: nc.dram_tensor(f"w{g}", [DIN, HSH], MM_DT, kind="ExternalInput").ap()
          for g in ("f", "i", "h")}
    # packed per-partition scalars: [b_f | b_i | b_h | b_h+0.5], each (128, IT)
    biases = nc.dram_tensor("biases", [P, 4 * IT], f32, kind="ExternalInput").ap()
    out = nc.dram_tensor("out", [HSH, T], f32, kind="ExternalOutput").ap()

    # DRAM views: (KT*P, n) -> [p, k, n]
    xT_v = xT.rearrange("(k p) t -> p k t", p=P)
    w_v = {g: w.rearrange("(k p) h -> p k h", p=P) for g, w in wd.items()}

    with tile.TileContext(nc) as tc:
        with (
            tc.tile_pool(name="wpool", bufs=1) as wpool,
            tc.tile_pool(name="bpool", bufs=1) as bpool,
            tc.tile_pool(name="xpool", bufs=8) as xpool,
            tc.tile_pool(name="psum", bufs=8, space="PSUM") as pspool,
            tc.tile_pool(name="work", bufs=4) as work,
            tc.tile_pool(name="hpool", bufs=6) as hpool,
        ):
            bias_t = bpool.tile([P, 4 * IT], f32, tag="bias")

            # all W per-k: consumption is k-ordered, so per-k 128KB tiles
            # keep the DMA->matmul dependency just-in-time (a batched 1MB
            # descriptor completes after its first consumer needs it)
            wt = {g: [wpool.tile([P, HSH], MM_DT, tag=f"w{g}{k}", name=f"w{g}{k}_t")
                      for k in range(KT)] for g in ("f", "i", "h")}

            def w_ap(g, k, hsl):
                return wt[g][k][:, hsl]

            def bias_ap(kind, i):
                return bias_t[:, kind * IT + i:kind * IT + i + 1]

            def fchain(sf, si, ne):
                """Normalize: s=sf+si, r=1/s, f=sf*r. In-place: sf becomes f.

                The add goes to GPSIMD only on big chunks: it is the
                dependency head (fed straight by the ACT sigmoids), so its
                software latency hides; everything downstream of the
                reciprocal stays on the DVE.
                """
                s = work.tile([P, ne], f32, tag="s", name="s_t")
                eng = nc.gpsimd if ne >= 1024 else nc.vector
                eng.tensor_add(s[:], sf[:, :ne], si[:, :ne])
                r = work.tile([P, ne], f32, tag="r", name="r_t")
                nc.vector.reciprocal_approx_fast(out=r[:], in_=s[:])
                nc.vector.tensor_mul(sf[:, :ne], sf[:, :ne], r[:])

            def hchain(i, f, gt, J, t0, ne):
                """mv=(f-1)*g, scan state=f*state-mv, store chunk."""
                nc.vector.scalar_tensor_tensor(
                    gt[:, :ne], f[:, :ne], 1.0, gt[:, :ne],
                    op0=OP.subtract, op1=OP.mult)
                hc = hpool.tile([P, ne], f32, tag="h", name=f"h{i}_t")
                init = 1.0 if J == 0 else hprev[i][:, -1:]
                nc.vector.tensor_tensor_scan(
                    hc[:], f[:, :ne], gt[:, :ne], init,
                    op0=OP.mult, op1=OP.subtract)
                hprev[i] = hc
                nc.sync.dma_start(
                    out=out[i * P:(i + 1) * P, t0:t0 + ne], in_=hc[:])

            hprev = [None] * IT
            hsls = [slice(i * P, (i + 1) * P) for i in range(IT)]

            # Warmups on a zeroed scratch tile keep PE duty high from
            # preamble end until the first real data lands.
            scratch = bpool.tile([P, NT], MM_DT, tag="scratch")
            nc.vector.memset(scratch[:].bitcast(mybir.dt.uint16), 0)
            pswarm = pspool.tile([P, NT], f32, tag="ps", name="pswarm_t")
            for _ in range(N_WARM):
                nc.tensor.matmul(pswarm[:], lhsT=scratch[:, :P], rhs=scratch[:],
                                 start=True, stop=True)

            # ---- DMA issue: ALL early DMAs on the sync queue in strict
            # priority order. Issue costs ~605ns/descriptor and the early
            # hardware DMA bandwidth is scarce: anything issued from another
            # engine queue (or out of order) steals bandwidth from the
            # (x, W_f) stream the first matmuls wait on and delays the HAM
            # clock ramp by several us.
            xk0 = [xpool.tile([P, NT], MM_DT, tag="xk", name="xk_t", bufs=16)
                   for _ in range(KT)]
            for k in range(KT):
                nc.sync.dma_start(out=xk0[k][:], in_=xT_v[:, k, 0:NT])
                nc.sync.dma_start(out=wt["f"][k][:], in_=w_v["f"][:, k, :])
                if k == 0:
                    nc.sync.dma_start(out=bias_t[:], in_=biases[:])
            xk1 = [xpool.tile([P, NT], MM_DT, tag="xk", name="xk_t", bufs=16)
                   for _ in range(KT)]
            for k in range(KT):
                nc.sync.dma_start(out=xk1[k][:], in_=xT_v[:, k, NT:2 * NT])
            for g in ("i", "h"):
                for k in range(KT):
                    nc.sync.dma_start(out=wt[g][k][:], in_=w_v[g][:, k, :])

            def rhs0(k):
                return xk0[k][:]

            def rhs1(k):
                return xk1[k][:]

            # ---- J0: gate-major, k-outer; PE chases the DMA stream ----
            t0, ne = CHUNKS[0]
            sf = [work.tile([P, ne], f32, tag="sf", name="sf_t") for _ in range(IT)]
            si = [work.tile([P, ne], f32, tag="si", name="si_t") for _ in range(IT)]
            sg = [work.tile([P, ne], f32, tag="sg", name="sg_t") for _ in range(IT)]
            gt = [work.tile([P, ne], f32, tag="gt", name="gt_t") for _ in range(IT)]
            for gate, dsts, bk in (("f", sf, 0), ("i", si, 1), ("h", sg, 2)):
                for half, rhsf in enumerate((rhs0, rhs1)):
                    esl = slice(half * NT, (half + 1) * NT)
                    psts = [pspool.tile([P, NT], f32, tag="ps", name="ps_t")
                            for _ in range(IT)]
                    for k in range(KT):
                        for pst, hsl in zip(psts, hsls):
                            nc.tensor.matmul(
                                pst[:], lhsT=w_ap(gate, k, hsl), rhs=rhsf(k),
                                start=(k == 0), stop=(k == KT - 1))
                    for i in range(IT):
                        nc.scalar.activation(dsts[i][:, esl], psts[i][:], AF.Sigmoid,
                                             bias=bias_ap(bk, i), scale=1.0)
                        if gate == "h":
                            # DVE: GPSIMD cannot read PSUM
                            nc.vector.scalar_tensor_tensor(
                                gt[i][:, esl], psts[i][:], bias_ap(3, i),
                                sg[i][:, esl], op0=OP.add, op1=OP.max)
            for i in range(IT):
                fchain(sf[i], si[i], ne)
                hchain(i, sf[i], gt[i], 0, t0, ne)

            # ---- J1+: h-tile-major units ----
            def load_xcs(t0, ne):
                """One batched [P, KT*w] tile (single descriptor) per
                column group."""
                nfull, rem = divmod(ne, NT)
                xcs = []
                toff = t0
                for w_ in [NT] * nfull + ([rem] if rem else []):
                    xt = xpool.tile([P, KT * w_], MM_DT, tag="x8", name="x8_t",
                                    bufs=4)
                    nc.sync.dma_start(out=xt[:], in_=xT_v[:, :, toff:toff + w_])
                    xcs.append((xt, toff - t0, w_))
                    toff += w_
                return xcs

            def mm_group(gate, hsl, xt, w_):
                """K-accumulated matmul group -> PSUM tile (full bank: PSUM
                zeroing on start=True is 2KB-bank granular)."""
                pst = pspool.tile([P, NT], f32, tag="ps", name="ps_t")
                for k in range(KT):
                    nc.tensor.matmul(
                        pst[:, :w_], lhsT=w_ap(gate, k, hsl),
                        rhs=xt[:, k * w_:(k + 1) * w_],
                        start=(k == 0), stop=(k == KT - 1))
                return pst

            def unit_f(i, ne, xcs):
                """f/i gates + normalization for one (h-tile, chunk)."""
                sf = work.tile([P, ne], f32, tag="sf", name="sf_t")
                si = work.tile([P, ne], f32, tag="si", name="si_t")
                for xt, eoff, w_ in xcs:
                    esl = slice(eoff, eoff + w_)
                    for gate, dst, bk in (("f", sf, 0), ("i", si, 1)):
                        pst = mm_group(gate, hsls[i], xt, w_)
                        nc.scalar.activation(
                            dst[:, esl], pst[:, :w_], AF.Sigmoid,
                            bias=bias_ap(bk, i), scale=1.0)
                fchain(sf, si, ne)
                return sf

            def unit_h(i, J, t0, ne, xcs, sf):
                """h gate, g-max, scan, store for one (h-tile, chunk)."""
                sg = work.tile([P, ne], f32, tag="sg", name="sg_t")
                gt = work.tile([P, ne], f32, tag="gt", name="gt_t")
                for xt, eoff, w_ in xcs:
                    esl = slice(eoff, eoff + w_)
                    pst = mm_group("h", hsls[i], xt, w_)
                    nc.scalar.activation(
                        sg[:, esl], pst[:, :w_], AF.Sigmoid,
                        bias=bias_ap(2, i), scale=1.0)
                    # DVE: GPSIMD cannot read PSUM
                    nc.vector.scalar_tensor_tensor(
                        gt[:, esl], pst[:, :w_], bias_ap(3, i),
                        sg[:, esl], op0=OP.add, op1=OP.max)
                hchain(i, sf, gt, J, t0, ne)

            def unit(i, J, t0, ne, xcs):
                unit_h(i, J, t0, ne, xcs, unit_f(i, ne, xcs))

            # middle 1024-chunks: h-tile-major
            for J, (t0, ne) in enumerate(CHUNKS[1:-2], start=1):
                xcs = load_xcs(t0, ne)
                for i in range(IT):
                    unit(i, J, t0, ne, xcs)

            # final two 512-chunks, paired per h-tile so the end-of-kernel
            # drain is a single unit's h-chain; the very last unit's f-phase
            # is hoisted ahead of its sibling's h-phase
            JA, JB = len(CHUNKS) - 2, len(CHUNKS) - 1
            (tA, neA), (tB, neB) = CHUNKS[JA], CHUNKS[JB]
            xcsA = load_xcs(tA, neA)
            xcsB = load_xcs(tB, neB)
            for i in range(IT - 1):
                unit(i, JA, tA, neA, xcsA)
                unit(i, JB, tB, neB, xcsB)
            iL = IT - 1
            sfA = unit_f(iL, neA, xcsA)
            sfB = unit_f(iL, neB, xcsB)
            unit_h(iL, JA, tA, neA, xcsA, sfA)
            unit_h(iL, JB, tB, neB, xcsB, sfB)

    nc.compile()
    return nc


def _in_maps(x, W_f, b_f, W_i, b_i, W_h, b_h):
    x = np.asarray(x, np.float32)
    wT = {g: np.ascontiguousarray(np.asarray(w, np.float32).T).astype(NP_MM_DT)
          for g, w in (("f", W_f), ("i", W_i), ("h", W_h))}
    bs = {g: np.asarray(b, np.float32) for g, b in (("f", b_f), ("i", b_i), ("h", b_h))}

    maps = []
    for c in range(N_CORES):
        b, hh = divmod(c, 2)
        hsl = slice(hh * HSH, (hh + 1) * HSH)
        bias_pack = np.concatenate([
            bs["f"][hsl].reshape(IT, P).T,
            bs["i"][hsl].reshape(IT, P).T,
            bs["h"][hsl].reshape(IT, P).T,
            (bs["h"][hsl] + 0.5).reshape(IT, P).T,
        ], axis=1)
        maps.append({
            "xT": np.ascontiguousarray(x[b].T).astype(NP_MM_DT),
            "wf": np.ascontiguousarray(wT["f"][:, hsl]),
            "wi": np.ascontiguousarray(wT["i"][:, hsl]),
            "wh": np.ascontiguousarray(wT["h"][:, hsl]),
            "biases": np.ascontiguousarray(bias_pack, dtype=np.float32),
        })
    return maps


def kernel(x, W_f, b_f, W_i, b_i, W_h, b_h):
    global _COMPILED
    if _COMPILED is None:
        _COMPILED = _build()
    nc = _COMPILED

    res = run_bass_kernel_spmd(
        nc, _in_maps(x, W_f, b_f, W_i, b_i, W_h, b_h), list(range(N_CORES)))

    full = np.empty((B, T, DH), np.float32)
    for c in range(N_CORES):
        b, hh = divmod(c, 2)
        full[b, :, hh * HSH:(hh + 1) * HSH] = res.results[c]["out"].T
    return full
